# revision 11
# baseline (speedup 1.0000x reference)
"""GAT (3-layer, 6/6/1 heads) + MLP classifier on Trainium2, 8 NeuronCores.

v4: destination-node partition (each core owns N/8 = 6250 dst nodes and all
edges into them). Node rows live in an "AB" row space: row = owner*3125 +
(local<3125 ? local : 25000 + local-3125), so each half-table (A = rows
0..24999, B = rows 25000..49999) is the concatenation of every core's half
shard and is AllGather-able as one contiguous tensor while keeping gather
indices within int16 range for the batched SWDGE dma_gather instruction.

Per layer: project own shard (h | alpha_src | alpha_dst extended weights,
bf16 matmuls) into shardA/shardB; AllGather A (issued mid-loop) then B into
tableA/tableB (rows padded to a 256B multiple for dma_gather); per dst tile
one batched dma_gather per half-window pulls all of the tile's source rows;
attention softmax runs as wide batched vector ops; bf16 mask matmuls
scatter-add messages with the softmax denominator riding along as extra
matmul columns. alpha_dst per edge comes from PE (transposed 0/1 dst mask
matmul against the tile's alpha_dst values, kept in SBUF since projection).
"""

import sys

sys.path.insert(0, "/opt/trn_rl_repo")

import numpy as np

from concourse import bass, mybir, tile, bacc
from concourse import bass_utils

P = 128
N, E, F_IN, C, H, N_CLS = 50000, 200000, 256, 128, 6, 40
HC = H * C  # 768
NCORES = 8
NPC = N // NCORES          # 6250
HALF = NPC // 2            # 3125
SPLIT = NCORES * HALF      # 25000 rows in table A
TPC = (NPC + P - 1) // P   # 49
BN_EPS = 1e-5
NEG_SLOPE = 0.2

ROW2 = 896   # table row cols for layers 1,2 (bf16 1792B, 256B multiple)
ROW3 = 256   # layer 3 (512B)

F32 = mybir.dt.float32
BF16 = mybir.dt.bfloat16
I16 = mybir.dt.int16
TDT = BF16
AX = mybir.AxisListType
ALU = mybir.AluOpType
ACTF = mybir.ActivationFunctionType


# ---------------------------------------------------------------- host prep
def _rows_of_nodes(nodes):
    k = nodes // NPC
    j = nodes % NPC
    return np.where(j < HALF, k * HALF + j, SPLIT + k * HALF + (j - HALF))


def _edge_arrays(edge_index):
    """Slot layout shared by all 3 layers: per (core, dst tile) the slots are
    [A-window run | B-window run], each padded to 128-multiples; chunk counts
    maxed over cores (SPMD single program).

    Returns (idxT [ncores,128,8*n_chunks] i16, dlocT [ncores,P,n_chunks] f32,
    Klo [tpc], Khi [tpc])."""
    src = np.asarray(edge_index[0], np.int64)
    dst = np.asarray(edge_index[1], np.int64)
    order = np.argsort(dst, kind="stable")
    src_s = src[order]
    dst_s = dst[order]
    rows = _rows_of_nodes(src_s)

    core = dst_s // NPC
    t = (dst_s % NPC) // P
    w = (rows >= SPLIT).astype(np.int64)  # 0 = A window, 1 = B window
    block = (core * TPC + t) * 2 + w
    nblocks = NCORES * TPC * 2
    counts = np.bincount(block, minlength=nblocks).reshape(NCORES, TPC, 2)
    cmax = counts.max(axis=0)                   # [TPC, 2] max slots per window
    cmax = np.maximum(cmax, 1)
    nidx = -(-cmax // 16) * 16                  # gather counts (16-multiple)
    kcnt = -(-cmax // P)                        # chunks per window
    Klo, Khi = kcnt[:, 0], kcnt[:, 1]
    Nlo, Nhi = nidx[:, 0], nidx[:, 1]
    K = Klo + Khi
    coff = np.concatenate([[0], np.cumsum(K)])  # chunk offset per tile
    n_chunks = int(K.sum())

    # chunk column of window w of tile t starts at coff[t] (+Klo[t] if w=1)
    wbase = np.stack([coff[:-1], coff[:-1] + Klo], axis=1)  # [TPC, 2]

    order2 = np.argsort(block, kind="stable")
    src_b = rows[order2]
    dst_b = dst_s[order2]
    blk = block[order2]
    starts = np.searchsorted(blk, np.arange(nblocks))
    j = np.arange(len(blk)) - starts[blk]       # rank within (core,tile,win)

    cc = core[order2]
    tt = t[order2]
    ww = w[order2]
    chunk = wbase[tt, ww] + j // P              # global chunk col
    part = j % P

    dlocT = np.full((NCORES, P, n_chunks), 200.0, np.float32)
    dlocT[cc, part, chunk] = (dst_b - cc * NPC - tt * P).astype(np.float32)

    # idx values relative to window base
    idxval = (src_b - ww * SPLIT).astype(np.int16)
    # flat position within window = jw; idx col = 8*wbase + jw//16, row jw%16
    idx16 = np.zeros((NCORES, 16, 8 * n_chunks), np.int16)
    idx16[cc, j % 16, 8 * wbase[tt, ww] + j // 16] = idxval
    idxT = np.tile(idx16, (1, 8, 1))            # replicate across Q7 groups
    return (idxT, dlocT, Klo.astype(int), Khi.astype(int),
            Nlo.astype(int), Nhi.astype(int))


def _wext(Wnp, a_s, a_d):
    heads, cout = a_s.shape
    A_s = np.zeros((heads * cout, heads), np.float32)
    A_d = np.zeros((heads * cout, heads), np.float32)
    for h in range(heads):
        A_s[h * cout : (h + 1) * cout, h] = a_s[h]
        A_d[h * cout : (h + 1) * cout, h] = a_d[h]
    return np.concatenate([Wnp, Wnp @ A_s, Wnp @ A_d], axis=1).astype(np.float32)


def _bn_cols(g, be, b):
    inv = 1.0 / np.sqrt(1.0 + BN_EPS)
    scale = (g * inv).astype(np.float32)
    shift = (b * scale + be).astype(np.float32)
    nh = len(g) // P
    return scale.reshape(nh, P).T.copy(), shift.reshape(nh, P).T.copy()


# ------------------------------------------------------------- bass program
def _build(Klo, Khi, Nlo, Nhi):
    K = Klo + Khi
    coff = np.concatenate([[0], np.cumsum(K)])
    n_chunks = int(K.sum())
    nc = bacc.Bacc("TRN2", target_bir_lowering=False, debug=False,
                   num_devices=NCORES)

    def din(name, shape, dt=F32):
        return nc.dram_tensor(name, shape, dt, kind="ExternalInput").ap()

    xT = din("xT", [F_IN, NPC], BF16)
    idxT = din("idxT", [P, 8 * n_chunks], I16)
    dlocT = din("dlocT", [P, n_chunks])
    W1e = din("W1e", [F_IN, HC + 12], BF16)
    W2e = din("W2e", [HC, HC + 12], BF16)
    W3e = din("W3e", [HC, C + 2], BF16)
    sc1 = din("sc1", [P, H]); sh1 = din("sh1", [P, H])
    sc2 = din("sc2", [P, H]); sh2 = din("sh2", [P, H])
    sc3 = din("sc3", [P, 1]); sh3 = din("sh3", [P, 1])
    Wc1 = din("Wc1", [C, C // 2])
    bc1 = din("bc1", [C // 2, 1])
    Wc2 = din("Wc2", [C // 2, N_CLS])
    bc2 = din("bc2", [N_CLS, 1])
    out = nc.dram_tensor("out", [NPC, N_CLS], F32, kind="ExternalOutput").ap()

    rg = [list(range(NCORES))]

    with tile.TileContext(nc) as tc:
        from contextlib import ExitStack
        ctx = ExitStack()
        cons = ctx.enter_context(tc.tile_pool(name="cons", bufs=1))
        dram = ctx.enter_context(tc.tile_pool(name="dram", bufs=1, space="DRAM"))
        big = ctx.enter_context(tc.tile_pool(name="big", bufs=4))
        sb = ctx.enter_context(tc.tile_pool(name="sb", bufs=4))
        ep = ctx.enter_context(tc.tile_pool(name="ep", bufs=3))
        ps_agg = ctx.enter_context(tc.tile_pool(name="ps_agg", bufs=2, space="PSUM"))
        ps_tp = ctx.enter_context(tc.tile_pool(name="ps_tp", bufs=1, space="PSUM"))
        ps_pj = ctx.enter_context(tc.tile_pool(name="ps_pj", bufs=1, space="PSUM"))

        def load_const(ap_in, shape, dt=F32, name="c"):
            t = cons.tile(shape, dt, name=name)
            nc.sync.dma_start(out=t[:], in_=ap_in)
            return t

        w1_sb = [load_const(W1e[k * P:(k + 1) * P, :], [P, HC + 12], BF16,
                            name=f"w1_{k}") for k in range(F_IN // P)]
        w2_sb = [load_const(W2e[k * P:(k + 1) * P, :], [P, HC + 12], BF16,
                            name=f"w2_{k}") for k in range(HC // P)]
        w3_sb = [load_const(W3e[k * P:(k + 1) * P, :], [P, C + 2], BF16,
                            name=f"w3_{k}") for k in range(HC // P)]
        sc1_sb = load_const(sc1[:, :], [P, H], name="sc1")
        sh1_sb = load_const(sh1[:, :], [P, H], name="sh1")
        sc2_sb = load_const(sc2[:, :], [P, H], name="sc2")
        sh2_sb = load_const(sh2[:, :], [P, H], name="sh2")
        sc3_sb = load_const(sc3[:, :], [P, 1], name="sc3")
        sh3_sb = load_const(sh3[:, :], [P, 1], name="sh3")
        wc1_sb = load_const(Wc1[:, :], [C, C // 2], name="wc1")
        bc1_sb = load_const(bc1[:, :], [C // 2, 1], name="bc1")
        wc2_sb = load_const(Wc2[:, :], [C // 2, N_CLS], name="wc2")
        bc2_sb = load_const(bc2[:, :], [N_CLS, 1], name="bc2")
        idx_sb = load_const(idxT[:, :], [P, 8 * n_chunks], I16, name="idxsb")
        dloc_sb = load_const(dlocT[:, :], [P, n_chunks], name="dlocsb")

        iota_i = cons.tile([P, P], mybir.dt.int32, name="iota_i")
        nc.gpsimd.iota(iota_i[:], pattern=[[1, P]], base=0, channel_multiplier=0)
        iota_f = cons.tile([P, P], F32, name="iota_f")
        nc.vector.tensor_copy(out=iota_f[:], in_=iota_i[:])
        ident = cons.tile([P, P], F32, name="ident")
        from concourse.masks import make_identity
        make_identity(nc, ident[:])
        ident_bf = cons.tile([P, P], BF16, name="ident_bf")
        nc.vector.tensor_copy(out=ident_bf[:], in_=ident[:])

        # zero the hs rotation buffers once: slots skipped by exact-count
        # gathers must hold finite bf16, never uninitialized SBUF
        kmax = int(K.max()) + 1
        for _ in range(3):
            hs0 = big.tile([P, kmax, ROW2], TDT, tag="hs")
            nc.vector.memset(hs0[:].rearrange("p k g -> p (k g)"), 0.0)

        # alpha_dst per dst-local row, per layer, kept in SBUF (bf16)
        adt_all = [cons.tile([P, TPC * H], TDT, name="adt1"),
                   cons.tile([P, TPC * H], TDT, name="adt2"),
                   cons.tile([P, TPC * 1], TDT, name="adt3")]
        for a in adt_all:
            nc.vector.memset(a[:], 0.0)

        # ---- DRAM intermediates (A/B halves for split AllGather)
        shardA = [dram.tile([HALF, ROW2], TDT, name="shardA1"),
                  dram.tile([HALF, ROW2], TDT, name="shardA2"),
                  dram.tile([HALF, ROW3], TDT, name="shardA3")]
        shardB = [dram.tile([HALF, ROW2], TDT, name="shardB1"),
                  dram.tile([HALF, ROW2], TDT, name="shardB2"),
                  dram.tile([HALF, ROW3], TDT, name="shardB3")]
        tableA = [dram.tile([SPLIT, ROW2], TDT, addr_space="Shared", name="tableA1"),
                  dram.tile([SPLIT, ROW2], TDT, addr_space="Shared", name="tableA2"),
                  dram.tile([SPLIT, ROW3], TDT, addr_space="Shared", name="tableA3")]
        tableB = [dram.tile([SPLIT, ROW2], TDT, addr_space="Shared", name="tableB1"),
                  dram.tile([SPLIT, ROW2], TDT, addr_space="Shared", name="tableB2"),
                  dram.tile([SPLIT, ROW3], TDT, addr_space="Shared", name="tableB3")]

        def nt_of(t):
            return min(P, NPC - t * P)

        def shard_write(li, t, hp, nt, hvalid):
            """Write hp[:nt, :] rows t*P..t*P+nt into shardA/shardB halves."""
            r0 = t * P
            r1 = t * P + nt
            if r1 <= HALF:
                nc.sync.dma_start(out=shardA[li][r0:r1, :], in_=hp[:nt, :])
            elif r0 >= HALF:
                nc.sync.dma_start(out=shardB[li][r0 - HALF:r1 - HALF, :],
                                  in_=hp[:nt, :])
            else:
                m = HALF - r0
                nc.sync.dma_start(out=shardA[li][r0:HALF, :], in_=hp[:m, :])
                nc.sync.dma_start(out=shardB[li][0:r1 - HALF, :], in_=hp[m:nt, :])

        def project(li, t, lhs_tiles):
            """Project tile t of layer li+1's input; li = target layer index
            0,1,2. Writes shard halves + adt_all[li]."""
            nt = nt_of(t)
            w_sb = [w1_sb, w2_sb, w3_sb][li]
            wid = HC + 12 if li < 2 else C + 2
            hvalid = HC + 6 if li < 2 else C + 1
            rowp = ROW2 if li < 2 else ROW3
            nheads = H if li < 2 else 1
            nk = len(w_sb)
            hp = ep.tile([P, rowp], TDT, tag="hp")
            if li < 2:
                pA = ps_pj.tile([P, 512], F32, tag="pjA")
                pB = ps_pj.tile([P, wid - 512], F32, tag="pjB")
                for k in range(nk):
                    nc.tensor.matmul(out=pA[:nt, :], lhsT=lhs_tiles[k][:, :nt],
                                     rhs=w_sb[k][:, 0:512],
                                     start=(k == 0), stop=(k == nk - 1))
                    nc.tensor.matmul(out=pB[:nt, :], lhsT=lhs_tiles[k][:, :nt],
                                     rhs=w_sb[k][:, 512:wid],
                                     start=(k == 0), stop=(k == nk - 1))
                nc.scalar.activation(out=hp[:nt, 0:512], in_=pA[:nt, :],
                                     func=ACTF.Copy)
                nc.vector.tensor_copy(out=hp[:nt, 512:hvalid],
                                      in_=pB[:nt, 0:hvalid - 512])
                nc.scalar.activation(
                    out=adt_all[li][:nt, t * nheads:(t + 1) * nheads],
                    in_=pB[:nt, hvalid - 512:wid - 512], func=ACTF.Copy)
            else:
                pA = ps_pj.tile([P, wid], F32, tag="pjA")
                for k in range(nk):
                    nc.tensor.matmul(out=pA[:nt, :], lhsT=lhs_tiles[k][:, :nt],
                                     rhs=w_sb[k][:, :],
                                     start=(k == 0), stop=(k == nk - 1))
                nc.scalar.activation(out=hp[:nt, 0:hvalid], in_=pA[:nt, 0:hvalid],
                                     func=ACTF.Copy)
                nc.scalar.activation(
                    out=adt_all[li][:nt, t:t + 1],
                    in_=pA[:nt, hvalid:wid], func=ACTF.Copy)
            shard_write(li, t, hp, nt, hvalid)

        def allgather(li, half):
            src = shardA[li] if half == 0 else shardB[li]
            dst = tableA[li] if half == 0 else tableB[li]
            nc.gpsimd.collective_compute(
                "AllGather", ALU.bypass, replica_groups=rg,
                ins=[src[:, :].opt()], outs=[dst[:, :].opt()])

        def aggregate(li, t, nheads):
            """Self-loop chunk (direct DMA, identity mask) + batched gathers
            + attention softmax + scatter for dst tile t of layer li."""
            nt = nt_of(t)
            hw = C * nheads
            rowp = ROW2 if li < 2 else ROW3
            kl, kh = int(Klo[t]), int(Khi[t])
            nl, nh = int(Nlo[t]), int(Nhi[t])
            kk = kl + kh + 1                      # incl. self chunk 0
            c0 = int(coff[t])
            hs = big.tile([P, kk, rowp], TDT, tag="hs")
            # self-loop chunk: own shard rows t*P..t*P+nt (no gather, no AG dep)
            r0, r1 = t * P, t * P + nt
            if r1 <= HALF:
                nc.sync.dma_start(out=hs[:nt, 0, :], in_=shardA[li][r0:r1, :])
            elif r0 >= HALF:
                nc.sync.dma_start(out=hs[:nt, 0, :],
                                  in_=shardB[li][r0 - HALF:r1 - HALF, :])
            else:
                m = HALF - r0
                nc.sync.dma_start(out=hs[:m, 0, :], in_=shardA[li][r0:HALF, :])
                nc.sync.dma_start(out=hs[m:nt, 0, :],
                                  in_=shardB[li][0:r1 - HALF, :])
            nc.gpsimd.dma_gather(
                out_ap=hs[:, 1:1 + kl, :], in_ap=tableA[li][:, :],
                idxs_ap=idx_sb[:, 8 * c0:8 * (c0 + kl)],
                num_idxs=kl * P, num_idxs_reg=kl * P, elem_size=rowp)
            nc.gpsimd.dma_gather(
                out_ap=hs[:, 1 + kl:kk, :], in_ap=tableB[li][:, :],
                idxs_ap=idx_sb[:, 8 * (c0 + kl):8 * (c0 + kk - 1)],
                num_idxs=kh * P, num_idxs_reg=kh * P, elem_size=rowp)
            mask = big.tile([P, (kk - 1) * P], TDT, tag="mask")
            nc.vector.tensor_tensor(
                out=mask[:].rearrange("p (k q) -> p k q", k=kk - 1),
                in0=dloc_sb[:, c0:c0 + kk - 1].unsqueeze(2).broadcast_to(
                    [P, kk - 1, P]),
                in1=iota_f[:].unsqueeze(1).broadcast_to([P, kk - 1, P]),
                op=ALU.is_equal)
            adt = adt_all[li][:, t * nheads:(t + 1) * nheads]
            ad_ps = ps_pj.tile([P, kk * nheads], F32, tag="pjB")
            nc.tensor.matmul(out=ad_ps[:, 0:nheads], lhsT=ident_bf[:],
                             rhs=adt, start=True, stop=True)
            for k in range(1, kk):
                mtp = ps_tp.tile([P, P], BF16, tag="tp")
                nc.tensor.transpose(out=mtp[:],
                                    in_=mask[:, (k - 1) * P:k * P],
                                    identity=ident_bf[:])
                mts = sb.tile([P, P], BF16, tag="mts")
                nc.scalar.activation(out=mts[:], in_=mtp[:], func=ACTF.Copy)
                nc.tensor.matmul(out=ad_ps[:, k * nheads:(k + 1) * nheads],
                                 lhsT=mts[:], rhs=adt, start=True, stop=True)
            # attention logits + leaky relu + exp, batched over all chunks
            ee = sb.tile([P, kk * nheads], F32, tag="ee")
            nc.vector.tensor_tensor(
                out=ee[:].rearrange("p (k h) -> p k h", k=kk),
                in0=hs[:, :, hw:hw + nheads],
                in1=ad_ps[:].rearrange("p (k h) -> p k h", k=kk), op=ALU.add)
            lk = sb.tile([P, kk * nheads], F32, tag="lk")
            nc.vector.tensor_scalar_mul(lk[:], ee[:], NEG_SLOPE)
            nc.vector.tensor_tensor(out=ee[:], in0=ee[:], in1=lk[:], op=ALU.max)
            w = sb.tile([P, kk * nheads], TDT, tag="w")
            nc.scalar.activation(out=w[:], in_=ee[:], func=ACTF.Exp)
            w3 = w[:].rearrange("p (k h) -> p k h", k=kk)
            gw = hw + nheads
            msg = big.tile([P, kk * gw], TDT, tag="msg")
            msg3 = msg[:].rearrange("p (k g) -> p k g", k=kk)
            nc.vector.tensor_tensor(
                out=msg3[:, :, 0:hw].rearrange("p k (h c) -> p k h c", h=nheads),
                in0=hs[:, :, 0:hw].rearrange("p k (h c) -> p k h c", h=nheads),
                in1=w3.unsqueeze(3).broadcast_to([P, kk, nheads, C]),
                op=ALU.mult)
            nc.vector.tensor_copy(out=msg3[:, :, hw:gw], in_=w3)
            if nheads > 1:
                pA = ps_agg.tile([P, 512], F32, tag="agA")
                pB = ps_agg.tile([P, hw + nheads - 512], F32, tag="agB")
            else:
                pA = ps_agg.tile([P, hw + 1], F32, tag="agA")
                pB = None
            for k in range(kk):
                st, sp = (k == 0), (k == kk - 1)
                lhsT = ident_bf[:] if k == 0 else mask[:, (k - 1) * P:k * P]
                if nheads > 1:
                    nc.tensor.matmul(out=pA[:, :], lhsT=lhsT,
                                     rhs=msg[:, k * gw:k * gw + 512],
                                     start=st, stop=sp)
                    nc.tensor.matmul(out=pB[:, :], lhsT=lhsT,
                                     rhs=msg[:, k * gw + 512:(k + 1) * gw],
                                     start=st, stop=sp)
                else:
                    nc.tensor.matmul(out=pA[:, :], lhsT=lhsT,
                                     rhs=msg[:, k * gw:(k + 1) * gw],
                                     start=st, stop=sp)
            return pA, pB

        # ================= Layer 1 projection (from input xT)
        for t in range(TPC):
            nt = nt_of(t)
            lhs = []
            for k in range(F_IN // P):
                lt = sb.tile([P, P], BF16, tag="xlhs")
                nc.sync.dma_start(out=lt[:, :nt],
                                  in_=xT[k * P:(k + 1) * P, t * P:t * P + nt])
                lhs.append(lt)
            project(0, t, lhs)
            if t == 24:
                allgather(0, 0)
        allgather(0, 1)

        # ============ Layers 1,2 aggregation (+ fused next-layer projection)
        for li in range(2):
            sc_sb = [sc1_sb, sc2_sb][li]
            sh_sb = [sh1_sb, sh2_sb][li]
            for t in range(TPC):
                nt = nt_of(t)
                pA, pB = aggregate(li, t, H)
                recip = ep.tile([P, H], F32, tag="recip")
                nc.vector.reciprocal(out=recip[:], in_=pB[:, 256:262])
                agg = ep.tile([P, HC], BF16, tag="agg")
                for h in range(4):
                    nc.scalar.activation(out=agg[:, h * C:(h + 1) * C],
                                         in_=pA[:, h * C:(h + 1) * C],
                                         func=ACTF.Copy,
                                         scale=recip[:, h:h + 1])
                for h in range(4, 6):
                    nc.scalar.activation(out=agg[:, h * C:(h + 1) * C],
                                         in_=pB[:, (h - 4) * C:(h - 3) * C],
                                         func=ACTF.Copy,
                                         scale=recip[:, h:h + 1])
                outT = ep.tile([P, HC], BF16, tag="outT")
                for h in range(H):
                    ptp = ps_tp.tile([P, P], BF16, tag="tp")
                    nc.tensor.transpose(out=ptp[:], in_=agg[:, h * C:(h + 1) * C],
                                        identity=ident_bf[:])
                    nc.scalar.activation(out=outT[:, h * C:(h + 1) * C], in_=ptp[:],
                                         func=ACTF.Relu, bias=sh_sb[:, h:h + 1],
                                         scale=sc_sb[:, h:h + 1])
                lhs = [outT[:, k * P:(k + 1) * P] for k in range(HC // P)]
                project(li + 1, t, lhs)
                if t == 24:
                    allgather(li + 1, 0)
            allgather(li + 1, 1)

        # ================= Layer 3 aggregation + classifier + log_softmax
        for t in range(TPC):
            nt = nt_of(t)
            pA, _ = aggregate(2, t, 1)
            recip = ep.tile([P, 1], F32, tag="recip3")
            nc.vector.reciprocal(out=recip[:], in_=pA[:, C:C + 1])
            agg = ep.tile([P, C], F32, tag="agg3")
            nc.scalar.activation(out=agg[:], in_=pA[:, 0:C], func=ACTF.Copy,
                                 scale=recip[:, 0:1])
            ptp = ps_tp.tile([P, P], F32, tag="tp")
            nc.tensor.transpose(out=ptp[:], in_=agg[:], identity=ident[:])
            y3 = ep.tile([P, P], F32, tag="y3")
            nc.vector.tensor_scalar(out=y3[:], in0=ptp[:], scalar1=sc3_sb[:, 0:1],
                                    scalar2=sh3_sb[:, 0:1], op0=ALU.mult, op1=ALU.add)
            z1p = ps_pj.tile([C // 2, P], F32, tag="pjA")
            nc.tensor.matmul(out=z1p[:, :nt], lhsT=wc1_sb[:], rhs=y3[:, :nt],
                             start=True, stop=True)
            z1 = ep.tile([C // 2, P], F32, tag="z1")
            nc.scalar.activation(out=z1[:, :nt], in_=z1p[:, :nt], func=ACTF.Relu,
                                 bias=bc1_sb[:, 0:1])
            lgp = ps_pj.tile([N_CLS, P], F32, tag="pjB")
            nc.tensor.matmul(out=lgp[:, :nt], lhsT=wc2_sb[:], rhs=z1[:, :nt],
                             start=True, stop=True)
            lgb = ep.tile([N_CLS, P], F32, tag="lgb")
            nc.vector.tensor_scalar(out=lgb[:, :nt], in0=lgp[:, :nt],
                                    scalar1=bc2_sb[:, 0:1], scalar2=None,
                                    op0=ALU.add)
            ptp2 = ps_tp.tile([P, N_CLS], F32, tag="tp")
            nc.tensor.transpose(out=ptp2[:nt, :], in_=lgb[:, :nt],
                                identity=ident[:N_CLS, :N_CLS])
            mx = ep.tile([P, 1], F32, tag="mx")
            nc.vector.reduce_max(out=mx[:nt, :], in_=ptp2[:nt, :], axis=AX.X)
            xs = ep.tile([P, N_CLS], F32, tag="xs")
            nc.vector.tensor_scalar(out=xs[:nt, :], in0=ptp2[:nt, :],
                                    scalar1=mx[:nt, 0:1], scalar2=None,
                                    op0=ALU.subtract)
            ex = ep.tile([P, N_CLS], F32, tag="ex")
            ssum = ep.tile([P, 1], F32, tag="ssum")
            nc.scalar.activation(out=ex[:nt, :], in_=xs[:nt, :], func=ACTF.Exp,
                                 accum_out=ssum[:nt, 0:1])
            lns = ep.tile([P, 1], F32, tag="lns")
            nc.scalar.activation(out=lns[:nt, :], in_=ssum[:nt, :], func=ACTF.Ln)
            fin = ep.tile([P, N_CLS], F32, tag="fin")
            nc.vector.tensor_scalar(out=fin[:nt, :], in0=xs[:nt, :],
                                    scalar1=lns[:nt, 0:1], scalar2=None,
                                    op0=ALU.subtract)
            nc.sync.dma_start(out=out[t * P:t * P + nt, :], in_=fin[:nt, :])
        ctx.close()

    nc.compile()
    return nc


_CACHE = {}


def _get_program(edge_index_bytes, edge_index):
    key = edge_index_bytes
    if key not in _CACHE:
        idxT, dlocT, Klo, Khi, Nlo, Nhi = _edge_arrays(edge_index.astype(np.int64))
        nc = _build(np.asarray(Klo), np.asarray(Khi), np.asarray(Nlo),
                    np.asarray(Nhi))
        _CACHE[key] = (nc, idxT, dlocT)
    return _CACHE[key]


def prepare(inputs):
    """Returns (nc, in_maps) for the given full inputs."""
    x = np.asarray(inputs["x"], np.float32)
    edge_index = np.asarray(inputs["edge_index"], np.int32)
    nc, idxT, dlocT = _get_program(edge_index.tobytes(), edge_index)

    import ml_dtypes
    bf = lambda a: np.asarray(a, np.float32).astype(ml_dtypes.bfloat16)

    W1e = _wext(np.asarray(inputs["W1"], np.float32), inputs["a1s"], inputs["a1d"])
    W2e = _wext(np.asarray(inputs["W2"], np.float32), inputs["a2s"], inputs["a2d"])
    W3e = _wext(np.asarray(inputs["W3"], np.float32), inputs["a3s"], inputs["a3d"])
    sc1, sh1 = _bn_cols(inputs["g1"], inputs["be1"], inputs["b1"])
    sc2, sh2 = _bn_cols(inputs["g2"], inputs["be2"], inputs["b2"])
    sc3, sh3 = _bn_cols(inputs["g3"], inputs["be3"], inputs["b3"])

    shared = {
        "W1e": bf(W1e), "W2e": bf(W2e), "W3e": bf(W3e),
        "sc1": sc1, "sh1": sh1, "sc2": sc2, "sh2": sh2,
        "sc3": sc3, "sh3": sh3,
        "Wc1": np.asarray(inputs["Wc1"], np.float32),
        "bc1": np.asarray(inputs["bc1"], np.float32).reshape(-1, 1),
        "Wc2": np.asarray(inputs["Wc2"], np.float32),
        "bc2": np.asarray(inputs["bc2"], np.float32).reshape(-1, 1),
    }
    in_maps = []
    for k in range(NCORES):
        m = dict(shared)
        m["xT"] = bf(np.ascontiguousarray(x[k * NPC:(k + 1) * NPC].T))
        m["idxT"] = idxT[k]
        m["dlocT"] = dlocT[k]
        in_maps.append(m)
    return nc, in_maps


def kernel(**inputs):
    nc, in_maps = prepare(inputs)
    res = bass_utils.run_bass_kernel_spmd(nc, in_maps, core_ids=list(range(NCORES)))
    outs = [res.results[k]["out"] for k in range(NCORES)]
    return np.concatenate(outs, axis=0).astype(np.float32)


# revision 12
# speedup vs baseline: 1.0312x; 1.0312x over previous
"""GAT (3-layer, 6/6/1 heads) + MLP classifier on Trainium2, 8 NeuronCores.

v4: destination-node partition (each core owns N/8 = 6250 dst nodes and all
edges into them). Node rows live in an "AB" row space: row = owner*3125 +
(local<3125 ? local : 25000 + local-3125), so each half-table (A = rows
0..24999, B = rows 25000..49999) is the concatenation of every core's half
shard and is AllGather-able as one contiguous tensor while keeping gather
indices within int16 range for the batched SWDGE dma_gather instruction.

Per layer: project own shard (h | alpha_src | alpha_dst extended weights,
bf16 matmuls) into shardA/shardB; AllGather A (issued mid-loop) then B into
tableA/tableB (rows padded to a 256B multiple for dma_gather); per dst tile
one batched dma_gather per half-window pulls all of the tile's source rows;
attention softmax runs as wide batched vector ops; bf16 mask matmuls
scatter-add messages with the softmax denominator riding along as extra
matmul columns. alpha_dst per edge comes from PE (transposed 0/1 dst mask
matmul against the tile's alpha_dst values, kept in SBUF since projection).
"""

import sys

sys.path.insert(0, "/opt/trn_rl_repo")

import numpy as np

from concourse import bass, mybir, tile, bacc
from concourse import bass_utils

P = 128
N, E, F_IN, C, H, N_CLS = 50000, 200000, 256, 128, 6, 40
HC = H * C  # 768
NCORES = 8
NPC = N // NCORES          # 6250
HALF = NPC // 2            # 3125
SPLIT = NCORES * HALF      # 25000 rows in table A
TPC = (NPC + P - 1) // P   # 49
BN_EPS = 1e-5
NEG_SLOPE = 0.2

ROW2 = 896   # table row cols for layers 1,2 (bf16 1792B, 256B multiple)
ROW3 = 256   # layer 3 (512B)

F32 = mybir.dt.float32
BF16 = mybir.dt.bfloat16
I16 = mybir.dt.int16
TDT = BF16
AX = mybir.AxisListType
ALU = mybir.AluOpType
ACTF = mybir.ActivationFunctionType


# ---------------------------------------------------------------- host prep
def _rows_of_nodes(nodes):
    k = nodes // NPC
    j = nodes % NPC
    return np.where(j < HALF, k * HALF + j, SPLIT + k * HALF + (j - HALF))


def _edge_arrays(edge_index):
    """Slot layout shared by all 3 layers: per (core, dst tile) the slots are
    [A-window run | B-window run], each padded to 128-multiples; chunk counts
    maxed over cores (SPMD single program).

    Returns (idxT [ncores,128,8*n_chunks] i16, dlocT [ncores,P,n_chunks] f32,
    Klo [tpc], Khi [tpc])."""
    src = np.asarray(edge_index[0], np.int64)
    dst = np.asarray(edge_index[1], np.int64)
    order = np.argsort(dst, kind="stable")
    src_s = src[order]
    dst_s = dst[order]
    rows = _rows_of_nodes(src_s)

    core = dst_s // NPC
    t = (dst_s % NPC) // P
    w = (rows >= SPLIT).astype(np.int64)  # 0 = A window, 1 = B window
    block = (core * TPC + t) * 2 + w
    nblocks = NCORES * TPC * 2
    counts = np.bincount(block, minlength=nblocks).reshape(NCORES, TPC, 2)
    cmax = counts.max(axis=0)                   # [TPC, 2] max slots per window
    cmax = np.maximum(cmax, 1)
    nidx = -(-cmax // 16) * 16                  # gather counts (16-multiple)
    kcnt = -(-cmax // P)                        # chunks per window
    Klo, Khi = kcnt[:, 0], kcnt[:, 1]
    Nlo, Nhi = nidx[:, 0], nidx[:, 1]
    K = Klo + Khi
    coff = np.concatenate([[0], np.cumsum(K)])  # chunk offset per tile
    n_chunks = int(K.sum())

    # chunk column of window w of tile t starts at coff[t] (+Klo[t] if w=1)
    wbase = np.stack([coff[:-1], coff[:-1] + Klo], axis=1)  # [TPC, 2]

    order2 = np.argsort(block, kind="stable")
    src_b = rows[order2]
    dst_b = dst_s[order2]
    blk = block[order2]
    starts = np.searchsorted(blk, np.arange(nblocks))
    j = np.arange(len(blk)) - starts[blk]       # rank within (core,tile,win)

    cc = core[order2]
    tt = t[order2]
    ww = w[order2]
    chunk = wbase[tt, ww] + j // P              # global chunk col
    part = j % P

    dlocT = np.full((NCORES, P, n_chunks), 200.0, np.float32)
    dlocT[cc, part, chunk] = (dst_b - cc * NPC - tt * P).astype(np.float32)

    # idx values relative to window base
    idxval = (src_b - ww * SPLIT).astype(np.int16)
    # flat position within window = jw; idx col = 8*wbase + jw//16, row jw%16
    idx16 = np.zeros((NCORES, 16, 8 * n_chunks), np.int16)
    idx16[cc, j % 16, 8 * wbase[tt, ww] + j // 16] = idxval
    idxT = np.tile(idx16, (1, 8, 1))            # replicate across Q7 groups
    return (idxT, dlocT, Klo.astype(int), Khi.astype(int),
            Nlo.astype(int), Nhi.astype(int))


def _wext(Wnp, a_s, a_d):
    heads, cout = a_s.shape
    A_s = np.zeros((heads * cout, heads), np.float32)
    A_d = np.zeros((heads * cout, heads), np.float32)
    for h in range(heads):
        A_s[h * cout : (h + 1) * cout, h] = a_s[h]
        A_d[h * cout : (h + 1) * cout, h] = a_d[h]
    return np.concatenate([Wnp, Wnp @ A_s, Wnp @ A_d], axis=1).astype(np.float32)


def _bn_cols(g, be, b):
    inv = 1.0 / np.sqrt(1.0 + BN_EPS)
    scale = (g * inv).astype(np.float32)
    shift = (b * scale + be).astype(np.float32)
    nh = len(g) // P
    return scale.reshape(nh, P).T.copy(), shift.reshape(nh, P).T.copy()


# ------------------------------------------------------------- bass program
def _build(Klo, Khi, Nlo, Nhi):
    K = Klo + Khi
    coff = np.concatenate([[0], np.cumsum(K)])
    n_chunks = int(K.sum())
    nc = bacc.Bacc("TRN2", target_bir_lowering=False, debug=False,
                   num_devices=NCORES)

    def din(name, shape, dt=F32):
        return nc.dram_tensor(name, shape, dt, kind="ExternalInput").ap()

    xT = din("xT", [F_IN, NPC], BF16)
    idxT = din("idxT", [P, 8 * n_chunks], I16)
    dlocT = din("dlocT", [P, n_chunks])
    W1e = din("W1e", [F_IN, HC + 12], BF16)
    W2e = din("W2e", [HC, HC + 12], BF16)
    W3e = din("W3e", [HC, C + 2], BF16)
    sc1 = din("sc1", [P, H]); sh1 = din("sh1", [P, H])
    sc2 = din("sc2", [P, H]); sh2 = din("sh2", [P, H])
    sc3 = din("sc3", [P, 1]); sh3 = din("sh3", [P, 1])
    Wc1 = din("Wc1", [C, C // 2])
    bc1 = din("bc1", [C // 2, 1])
    Wc2 = din("Wc2", [C // 2, N_CLS])
    bc2 = din("bc2", [N_CLS, 1])
    out = nc.dram_tensor("out", [NPC, N_CLS], F32, kind="ExternalOutput").ap()

    rg = [list(range(NCORES))]

    with tile.TileContext(nc) as tc:
        from contextlib import ExitStack
        ctx = ExitStack()
        cons = ctx.enter_context(tc.tile_pool(name="cons", bufs=1))
        dram = ctx.enter_context(tc.tile_pool(name="dram", bufs=1, space="DRAM"))
        big = ctx.enter_context(tc.tile_pool(name="big", bufs=3))
        sb = ctx.enter_context(tc.tile_pool(name="sb", bufs=4))
        ep = ctx.enter_context(tc.tile_pool(name="ep", bufs=3))
        ps_agg = ctx.enter_context(tc.tile_pool(name="ps_agg", bufs=2, space="PSUM"))
        ps_tp = ctx.enter_context(tc.tile_pool(name="ps_tp", bufs=1, space="PSUM"))
        ps_pj = ctx.enter_context(tc.tile_pool(name="ps_pj", bufs=1, space="PSUM"))

        def load_const(ap_in, shape, dt=F32, name="c"):
            t = cons.tile(shape, dt, name=name)
            nc.sync.dma_start(out=t[:], in_=ap_in)
            return t

        w1_sb = [load_const(W1e[k * P:(k + 1) * P, :], [P, HC + 12], BF16,
                            name=f"w1_{k}") for k in range(F_IN // P)]
        w2_sb = [load_const(W2e[k * P:(k + 1) * P, :], [P, HC + 12], BF16,
                            name=f"w2_{k}") for k in range(HC // P)]
        w3_sb = [load_const(W3e[k * P:(k + 1) * P, :], [P, C + 2], BF16,
                            name=f"w3_{k}") for k in range(HC // P)]
        sc1_sb = load_const(sc1[:, :], [P, H], name="sc1")
        sh1_sb = load_const(sh1[:, :], [P, H], name="sh1")
        sc2_sb = load_const(sc2[:, :], [P, H], name="sc2")
        sh2_sb = load_const(sh2[:, :], [P, H], name="sh2")
        sc3_sb = load_const(sc3[:, :], [P, 1], name="sc3")
        sh3_sb = load_const(sh3[:, :], [P, 1], name="sh3")
        wc1_sb = load_const(Wc1[:, :], [C, C // 2], name="wc1")
        bc1_sb = load_const(bc1[:, :], [C // 2, 1], name="bc1")
        wc2_sb = load_const(Wc2[:, :], [C // 2, N_CLS], name="wc2")
        bc2_sb = load_const(bc2[:, :], [N_CLS, 1], name="bc2")
        idx_sb = load_const(idxT[:, :], [P, 8 * n_chunks], I16, name="idxsb")
        dloc_sb = load_const(dlocT[:, :], [P, n_chunks], name="dlocsb")

        iota_i = cons.tile([P, P], mybir.dt.int32, name="iota_i")
        nc.gpsimd.iota(iota_i[:], pattern=[[1, P]], base=0, channel_multiplier=0)
        iota_f = cons.tile([P, P], F32, name="iota_f")
        nc.vector.tensor_copy(out=iota_f[:], in_=iota_i[:])
        ident = cons.tile([P, P], F32, name="ident")
        from concourse.masks import make_identity
        make_identity(nc, ident[:])
        ident_bf = cons.tile([P, P], BF16, name="ident_bf")
        nc.vector.tensor_copy(out=ident_bf[:], in_=ident[:])

        # zero the hs rotation buffers once: slots skipped by exact-count
        # gathers must hold finite bf16, never uninitialized SBUF
        kmax = int(K.max()) + 1
        for _ in range(3):
            hs0 = big.tile([P, kmax, ROW2], TDT, tag="hs")
            nc.vector.memset(hs0[:].rearrange("p k g -> p (k g)"), 0.0)

        # alpha_dst per dst-local row, per layer, kept in SBUF (bf16)
        adt_all = [cons.tile([P, TPC * H], TDT, name="adt1"),
                   cons.tile([P, TPC * H], TDT, name="adt2"),
                   cons.tile([P, TPC * 1], TDT, name="adt3")]
        for a in adt_all:
            nc.vector.memset(a[:], 0.0)

        # ---- DRAM intermediates (A/B halves for split AllGather)
        shardA = [dram.tile([HALF, ROW2], TDT, name="shardA1"),
                  dram.tile([HALF, ROW2], TDT, name="shardA2"),
                  dram.tile([HALF, ROW3], TDT, name="shardA3")]
        shardB = [dram.tile([HALF, ROW2], TDT, name="shardB1"),
                  dram.tile([HALF, ROW2], TDT, name="shardB2"),
                  dram.tile([HALF, ROW3], TDT, name="shardB3")]
        tableA = [dram.tile([SPLIT, ROW2], TDT, addr_space="Shared", name="tableA1"),
                  dram.tile([SPLIT, ROW2], TDT, addr_space="Shared", name="tableA2"),
                  dram.tile([SPLIT, ROW3], TDT, addr_space="Shared", name="tableA3")]
        tableB = [dram.tile([SPLIT, ROW2], TDT, addr_space="Shared", name="tableB1"),
                  dram.tile([SPLIT, ROW2], TDT, addr_space="Shared", name="tableB2"),
                  dram.tile([SPLIT, ROW3], TDT, addr_space="Shared", name="tableB3")]

        def nt_of(t):
            return min(P, NPC - t * P)

        def shard_write(li, t, hp, nt, hvalid):
            """Write hp[:nt, :] rows t*P..t*P+nt into shardA/shardB halves."""
            r0 = t * P
            r1 = t * P + nt
            if r1 <= HALF:
                nc.sync.dma_start(out=shardA[li][r0:r1, :], in_=hp[:nt, :])
            elif r0 >= HALF:
                nc.sync.dma_start(out=shardB[li][r0 - HALF:r1 - HALF, :],
                                  in_=hp[:nt, :])
            else:
                m = HALF - r0
                nc.sync.dma_start(out=shardA[li][r0:HALF, :], in_=hp[:m, :])
                nc.sync.dma_start(out=shardB[li][0:r1 - HALF, :], in_=hp[m:nt, :])

        def project(li, t, lhs_tiles):
            """Project tile t of layer li+1's input; li = target layer index
            0,1,2. Writes shard halves + adt_all[li]."""
            nt = nt_of(t)
            w_sb = [w1_sb, w2_sb, w3_sb][li]
            wid = HC + 12 if li < 2 else C + 2
            hvalid = HC + 6 if li < 2 else C + 1
            rowp = ROW2 if li < 2 else ROW3
            nheads = H if li < 2 else 1
            nk = len(w_sb)
            hp = ep.tile([P, rowp], TDT, tag="hp")
            if li < 2:
                pA = ps_pj.tile([P, 512], F32, tag="pjA")
                pB = ps_pj.tile([P, wid - 512], F32, tag="pjB")
                for k in range(nk):
                    nc.tensor.matmul(out=pA[:nt, :], lhsT=lhs_tiles[k][:, :nt],
                                     rhs=w_sb[k][:, 0:512],
                                     start=(k == 0), stop=(k == nk - 1))
                    nc.tensor.matmul(out=pB[:nt, :], lhsT=lhs_tiles[k][:, :nt],
                                     rhs=w_sb[k][:, 512:wid],
                                     start=(k == 0), stop=(k == nk - 1))
                nc.scalar.activation(out=hp[:nt, 0:512], in_=pA[:nt, :],
                                     func=ACTF.Copy)
                nc.vector.tensor_copy(out=hp[:nt, 512:hvalid],
                                      in_=pB[:nt, 0:hvalid - 512])
                nc.scalar.activation(
                    out=adt_all[li][:nt, t * nheads:(t + 1) * nheads],
                    in_=pB[:nt, hvalid - 512:wid - 512], func=ACTF.Copy)
            else:
                pA = ps_pj.tile([P, wid], F32, tag="pjA")
                for k in range(nk):
                    nc.tensor.matmul(out=pA[:nt, :], lhsT=lhs_tiles[k][:, :nt],
                                     rhs=w_sb[k][:, :],
                                     start=(k == 0), stop=(k == nk - 1))
                nc.scalar.activation(out=hp[:nt, 0:hvalid], in_=pA[:nt, 0:hvalid],
                                     func=ACTF.Copy)
                nc.scalar.activation(
                    out=adt_all[li][:nt, t:t + 1],
                    in_=pA[:nt, hvalid:wid], func=ACTF.Copy)
            shard_write(li, t, hp, nt, hvalid)

        def allgather(li, half):
            src = shardA[li] if half == 0 else shardB[li]
            dst = tableA[li] if half == 0 else tableB[li]
            nc.gpsimd.collective_compute(
                "AllGather", ALU.bypass, replica_groups=rg,
                ins=[src[:, :].opt()], outs=[dst[:, :].opt()])

        def aggregate(li, t, nheads):
            """Self-loop chunk (direct DMA, identity mask) + batched gathers
            + attention softmax + scatter for dst tile t of layer li."""
            nt = nt_of(t)
            hw = C * nheads
            rowp = ROW2 if li < 2 else ROW3
            kl, kh = int(Klo[t]), int(Khi[t])
            nl, nh = int(Nlo[t]), int(Nhi[t])
            kk = kl + kh + 1                      # incl. self chunk 0
            c0 = int(coff[t])
            hs = big.tile([P, kk, rowp], TDT, tag="hs")
            # self-loop chunk: own shard rows t*P..t*P+nt (no gather, no AG dep)
            r0, r1 = t * P, t * P + nt
            if r1 <= HALF:
                nc.sync.dma_start(out=hs[:nt, 0, :], in_=shardA[li][r0:r1, :])
            elif r0 >= HALF:
                nc.sync.dma_start(out=hs[:nt, 0, :],
                                  in_=shardB[li][r0 - HALF:r1 - HALF, :])
            else:
                m = HALF - r0
                nc.sync.dma_start(out=hs[:m, 0, :], in_=shardA[li][r0:HALF, :])
                nc.sync.dma_start(out=hs[m:nt, 0, :],
                                  in_=shardB[li][0:r1 - HALF, :])
            nc.gpsimd.dma_gather(
                out_ap=hs[:, 1:1 + kl, :], in_ap=tableA[li][:, :],
                idxs_ap=idx_sb[:, 8 * c0:8 * (c0 + kl)],
                num_idxs=kl * P, num_idxs_reg=kl * P, elem_size=rowp)
            nc.gpsimd.dma_gather(
                out_ap=hs[:, 1 + kl:kk, :], in_ap=tableB[li][:, :],
                idxs_ap=idx_sb[:, 8 * (c0 + kl):8 * (c0 + kk - 1)],
                num_idxs=kh * P, num_idxs_reg=kh * P, elem_size=rowp)
            mask = big.tile([P, (kk - 1) * P], TDT, tag="mask")
            nc.vector.tensor_tensor(
                out=mask[:].rearrange("p (k q) -> p k q", k=kk - 1),
                in0=dloc_sb[:, c0:c0 + kk - 1].unsqueeze(2).broadcast_to(
                    [P, kk - 1, P]),
                in1=iota_f[:].unsqueeze(1).broadcast_to([P, kk - 1, P]),
                op=ALU.is_equal)
            adt = adt_all[li][:, t * nheads:(t + 1) * nheads]
            ad_ps = ps_pj.tile([P, kk * nheads], F32, tag="pjB")
            nc.tensor.matmul(out=ad_ps[:, 0:nheads], lhsT=ident_bf[:],
                             rhs=adt, start=True, stop=True)
            for k in range(1, kk):
                mtp = ps_tp.tile([P, P], BF16, tag="tp")
                nc.tensor.transpose(out=mtp[:],
                                    in_=mask[:, (k - 1) * P:k * P],
                                    identity=ident_bf[:])
                mts = sb.tile([P, P], BF16, tag="mts")
                nc.scalar.activation(out=mts[:], in_=mtp[:], func=ACTF.Copy)
                nc.tensor.matmul(out=ad_ps[:, k * nheads:(k + 1) * nheads],
                                 lhsT=mts[:], rhs=adt, start=True, stop=True)
            # attention logits + leaky relu + exp, batched over all chunks
            ee = sb.tile([P, kk * nheads], F32, tag="ee")
            nc.vector.tensor_tensor(
                out=ee[:].rearrange("p (k h) -> p k h", k=kk),
                in0=hs[:, :, hw:hw + nheads],
                in1=ad_ps[:].rearrange("p (k h) -> p k h", k=kk), op=ALU.add)
            lk = sb.tile([P, kk * nheads], F32, tag="lk")
            nc.vector.tensor_scalar_mul(lk[:], ee[:], NEG_SLOPE)
            nc.vector.tensor_tensor(out=ee[:], in0=ee[:], in1=lk[:], op=ALU.max)
            w = sb.tile([P, kk * nheads], TDT, tag="w")
            nc.scalar.activation(out=w[:], in_=ee[:], func=ACTF.Exp)
            w3 = w[:].rearrange("p (k h) -> p k h", k=kk)
            gw = hw + nheads
            msg = big.tile([P, kk * gw], TDT, tag="msg")
            msg3 = msg[:].rearrange("p (k g) -> p k g", k=kk)
            nc.vector.tensor_tensor(
                out=msg3[:, :, 0:hw].rearrange("p k (h c) -> p k h c", h=nheads),
                in0=hs[:, :, 0:hw].rearrange("p k (h c) -> p k h c", h=nheads),
                in1=w3.unsqueeze(3).broadcast_to([P, kk, nheads, C]),
                op=ALU.mult)
            nc.vector.tensor_copy(out=msg3[:, :, hw:gw], in_=w3)
            if nheads > 1:
                pA = ps_agg.tile([P, 512], F32, tag="agA")
                pB = ps_agg.tile([P, hw + nheads - 512], F32, tag="agB")
            else:
                pA = ps_agg.tile([P, hw + 1], F32, tag="agA")
                pB = None
            for k in range(kk):
                st, sp = (k == 0), (k == kk - 1)
                lhsT = ident_bf[:] if k == 0 else mask[:, (k - 1) * P:k * P]
                if nheads > 1:
                    nc.tensor.matmul(out=pA[:, :], lhsT=lhsT,
                                     rhs=msg[:, k * gw:k * gw + 512],
                                     start=st, stop=sp)
                    nc.tensor.matmul(out=pB[:, :], lhsT=lhsT,
                                     rhs=msg[:, k * gw + 512:(k + 1) * gw],
                                     start=st, stop=sp)
                else:
                    nc.tensor.matmul(out=pA[:, :], lhsT=lhsT,
                                     rhs=msg[:, k * gw:(k + 1) * gw],
                                     start=st, stop=sp)
            return pA, pB

        # ================= Layer 1 projection (from input xT)
        for t in range(TPC):
            nt = nt_of(t)
            lhs = []
            for k in range(F_IN // P):
                lt = sb.tile([P, P], BF16, tag="xlhs")
                nc.sync.dma_start(out=lt[:, :nt],
                                  in_=xT[k * P:(k + 1) * P, t * P:t * P + nt])
                lhs.append(lt)
            project(0, t, lhs)
            if t == 24:
                allgather(0, 0)
        allgather(0, 1)

        # ============ Layers 1,2 aggregation (+ fused next-layer projection)
        for li in range(2):
            sc_sb = [sc1_sb, sc2_sb][li]
            sh_sb = [sh1_sb, sh2_sb][li]
            for t in range(TPC):
                nt = nt_of(t)
                pA, pB = aggregate(li, t, H)
                recip = ep.tile([P, H], F32, tag="recip")
                nc.vector.reciprocal(out=recip[:], in_=pB[:, 256:262])
                agg = ep.tile([P, HC], BF16, tag="agg")
                for h in range(4):
                    nc.scalar.activation(out=agg[:, h * C:(h + 1) * C],
                                         in_=pA[:, h * C:(h + 1) * C],
                                         func=ACTF.Copy,
                                         scale=recip[:, h:h + 1])
                for h in range(4, 6):
                    nc.scalar.activation(out=agg[:, h * C:(h + 1) * C],
                                         in_=pB[:, (h - 4) * C:(h - 3) * C],
                                         func=ACTF.Copy,
                                         scale=recip[:, h:h + 1])
                outT = ep.tile([P, HC], BF16, tag="outT")
                for h in range(H):
                    ptp = ps_tp.tile([P, P], BF16, tag="tp")
                    nc.tensor.transpose(out=ptp[:], in_=agg[:, h * C:(h + 1) * C],
                                        identity=ident_bf[:])
                    nc.scalar.activation(out=outT[:, h * C:(h + 1) * C], in_=ptp[:],
                                         func=ACTF.Relu, bias=sh_sb[:, h:h + 1],
                                         scale=sc_sb[:, h:h + 1])
                lhs = [outT[:, k * P:(k + 1) * P] for k in range(HC // P)]
                project(li + 1, t, lhs)
                if t == 24:
                    allgather(li + 1, 0)
            allgather(li + 1, 1)

        # ================= Layer 3 aggregation + classifier + log_softmax
        for t in range(TPC):
            nt = nt_of(t)
            pA, _ = aggregate(2, t, 1)
            recip = ep.tile([P, 1], F32, tag="recip3")
            nc.vector.reciprocal(out=recip[:], in_=pA[:, C:C + 1])
            agg = ep.tile([P, C], F32, tag="agg3")
            nc.scalar.activation(out=agg[:], in_=pA[:, 0:C], func=ACTF.Copy,
                                 scale=recip[:, 0:1])
            ptp = ps_tp.tile([P, P], F32, tag="tp")
            nc.tensor.transpose(out=ptp[:], in_=agg[:], identity=ident[:])
            y3 = ep.tile([P, P], F32, tag="y3")
            nc.vector.tensor_scalar(out=y3[:], in0=ptp[:], scalar1=sc3_sb[:, 0:1],
                                    scalar2=sh3_sb[:, 0:1], op0=ALU.mult, op1=ALU.add)
            z1p = ps_pj.tile([C // 2, P], F32, tag="pjA")
            nc.tensor.matmul(out=z1p[:, :nt], lhsT=wc1_sb[:], rhs=y3[:, :nt],
                             start=True, stop=True)
            z1 = ep.tile([C // 2, P], F32, tag="z1")
            nc.scalar.activation(out=z1[:, :nt], in_=z1p[:, :nt], func=ACTF.Relu,
                                 bias=bc1_sb[:, 0:1])
            lgp = ps_pj.tile([N_CLS, P], F32, tag="pjB")
            nc.tensor.matmul(out=lgp[:, :nt], lhsT=wc2_sb[:], rhs=z1[:, :nt],
                             start=True, stop=True)
            lgb = ep.tile([N_CLS, P], F32, tag="lgb")
            nc.vector.tensor_scalar(out=lgb[:, :nt], in0=lgp[:, :nt],
                                    scalar1=bc2_sb[:, 0:1], scalar2=None,
                                    op0=ALU.add)
            ptp2 = ps_tp.tile([P, N_CLS], F32, tag="tp")
            nc.tensor.transpose(out=ptp2[:nt, :], in_=lgb[:, :nt],
                                identity=ident[:N_CLS, :N_CLS])
            mx = ep.tile([P, 1], F32, tag="mx")
            nc.vector.reduce_max(out=mx[:nt, :], in_=ptp2[:nt, :], axis=AX.X)
            xs = ep.tile([P, N_CLS], F32, tag="xs")
            nc.vector.tensor_scalar(out=xs[:nt, :], in0=ptp2[:nt, :],
                                    scalar1=mx[:nt, 0:1], scalar2=None,
                                    op0=ALU.subtract)
            ex = ep.tile([P, N_CLS], F32, tag="ex")
            ssum = ep.tile([P, 1], F32, tag="ssum")
            nc.scalar.activation(out=ex[:nt, :], in_=xs[:nt, :], func=ACTF.Exp,
                                 accum_out=ssum[:nt, 0:1])
            lns = ep.tile([P, 1], F32, tag="lns")
            nc.scalar.activation(out=lns[:nt, :], in_=ssum[:nt, :], func=ACTF.Ln)
            fin = ep.tile([P, N_CLS], F32, tag="fin")
            nc.vector.tensor_scalar(out=fin[:nt, :], in0=xs[:nt, :],
                                    scalar1=lns[:nt, 0:1], scalar2=None,
                                    op0=ALU.subtract)
            nc.sync.dma_start(out=out[t * P:t * P + nt, :], in_=fin[:nt, :])
        ctx.close()

    nc.compile()
    return nc


_CACHE = {}


def _get_program(edge_index_bytes, edge_index):
    key = edge_index_bytes
    if key not in _CACHE:
        idxT, dlocT, Klo, Khi, Nlo, Nhi = _edge_arrays(edge_index.astype(np.int64))
        nc = _build(np.asarray(Klo), np.asarray(Khi), np.asarray(Nlo),
                    np.asarray(Nhi))
        _CACHE[key] = (nc, idxT, dlocT)
    return _CACHE[key]


def prepare(inputs):
    """Returns (nc, in_maps) for the given full inputs."""
    x = np.asarray(inputs["x"], np.float32)
    edge_index = np.asarray(inputs["edge_index"], np.int32)
    nc, idxT, dlocT = _get_program(edge_index.tobytes(), edge_index)

    import ml_dtypes
    bf = lambda a: np.asarray(a, np.float32).astype(ml_dtypes.bfloat16)

    W1e = _wext(np.asarray(inputs["W1"], np.float32), inputs["a1s"], inputs["a1d"])
    W2e = _wext(np.asarray(inputs["W2"], np.float32), inputs["a2s"], inputs["a2d"])
    W3e = _wext(np.asarray(inputs["W3"], np.float32), inputs["a3s"], inputs["a3d"])
    sc1, sh1 = _bn_cols(inputs["g1"], inputs["be1"], inputs["b1"])
    sc2, sh2 = _bn_cols(inputs["g2"], inputs["be2"], inputs["b2"])
    sc3, sh3 = _bn_cols(inputs["g3"], inputs["be3"], inputs["b3"])

    shared = {
        "W1e": bf(W1e), "W2e": bf(W2e), "W3e": bf(W3e),
        "sc1": sc1, "sh1": sh1, "sc2": sc2, "sh2": sh2,
        "sc3": sc3, "sh3": sh3,
        "Wc1": np.asarray(inputs["Wc1"], np.float32),
        "bc1": np.asarray(inputs["bc1"], np.float32).reshape(-1, 1),
        "Wc2": np.asarray(inputs["Wc2"], np.float32),
        "bc2": np.asarray(inputs["bc2"], np.float32).reshape(-1, 1),
    }
    in_maps = []
    for k in range(NCORES):
        m = dict(shared)
        m["xT"] = bf(np.ascontiguousarray(x[k * NPC:(k + 1) * NPC].T))
        m["idxT"] = idxT[k]
        m["dlocT"] = dlocT[k]
        in_maps.append(m)
    return nc, in_maps


def kernel(**inputs):
    nc, in_maps = prepare(inputs)
    res = bass_utils.run_bass_kernel_spmd(nc, in_maps, core_ids=list(range(NCORES)))
    outs = [res.results[k]["out"] for k in range(NCORES)]
    return np.concatenate(outs, axis=0).astype(np.float32)
